# revision 17
# baseline (speedup 1.0000x reference)
"""Bass TRN2 kernel for the boundary cosine-similarity context loss.

Per core (8 cores): batch b = k//2, row-half h = k%2; slab = 194 rows
(h==0: global rows 0..193, h==1: 190..383); produced = slab rows 2..191.
19 macroblocks x 10 produced rows.

v2 layout: host supplies x as bf16 (padded); per block the 14-row g
window and its +1-shifted twin godd are DMA'd straight from HBM (no
gpsimd conversion, no inter-block copy chain). Dot products are chunked
[C, PRODW] on DVE feeding one-hot reduction matmuls in PSUM; norms are
ACT square + eye matmuls -> ln -> fused inv-product psum (identity +
5 dx-shift selection matmuls accumulate ln n2_p + ln n2_q) -> exp ->
ipq. Post: cos = pk*ipq; TTR-fused weighted MSE accumulation.
Host folds the per-batch 1/cnt, 1/24, valid, 1/n_valid scaling.
"""

import numpy as np
import ml_dtypes

import concourse.bass as bass
import concourse.mybir as mybir
from concourse.tile import TileContext
from concourse.vector_clock import ScopedClock
from concourse import bass_utils

BF16 = ml_dtypes.bfloat16
F32 = mybir.dt.float32
BF = mybir.dt.bfloat16

W = 384
C = 128
SHIFTS = [(0, 1), (0, 2),
          (1, -2), (1, -1), (1, 0), (1, 1), (1, 2),
          (2, -2), (2, -1), (2, 0), (2, 1), (2, 2)]
OFFS = [dy * W + dx for dy, dx in SHIFTS]
NSH = 12
RPB = 10
NROWS = NSH * RPB      # 120 packed rows
GW = 14 * W
GPAD = 8
XPAD = 16
PRODW = 384            # product chunk width (per DVE op)


def _patch_tile_drain():
    if getattr(TileContext, "_drain_patched", False):
        return

    def _drain_and_barrier(self, tick_clock, wait_clock):
        drain_inst = self.nc.sync.drain()
        wait_clock.add_sem_waits(
            drain_inst.ins, ScopedClock({None: tick_clock.global_clock}))
        si = drain_inst.ins.sync_info
        if si is not None and si.on_wait and len(si.on_wait) > 1:
            waits = list(si.on_wait)
            drain_inst.ins.sync_info = mybir.SyncInfo(
                on_wait=[waits[-1]], on_update=list(si.on_update or []))
            for w in waits[:-1]:
                nop = self.nc.sync.nop(nofuse=True)
                nop.ins.sync_info = mybir.SyncInfo(on_wait=[w], on_update=[])
        self.nc.all_engine_barrier()
        popped = self.nc._tile_sem_poison_stack.pop()
        assert popped is self._sem_poison
        self.nc.clear_and_free_semaphores(list(self.sems.allocated().values()))
        self.nc.all_engine_barrier()

    TileContext._drain_and_barrier = _drain_and_barrier
    TileContext._drain_patched = True


_WSPLIT_N = [0]


def _split_multi_waits(nc, max_waits=1):
    """This container's walrus rejects instructions with more than one sync
    wait; hoist excess waits onto same-engine NOPs inserted just before."""
    for fn in nc.m.functions:
        for blk in fn.blocks:
            insts = blk.instructions
            out = []
            for inst in insts:
                si = inst.sync_info
                if si is not None and si.on_wait and len(si.on_wait) > max_waits:
                    waits = list(si.on_wait)
                    keep = waits[-max_waits:]
                    for w in waits[:-max_waits]:
                        _WSPLIT_N[0] += 1
                        nop = mybir.InstNoOp(
                            name=f"wsplit_{_WSPLIT_N[0]}", ins=[], outs=[])
                        nop.engine = inst.engine
                        nop.sync_info = mybir.SyncInfo(on_wait=[w],
                                                       on_update=[])
                        out.append(nop)
                    inst.sync_info = mybir.SyncInfo(
                        on_wait=keep, on_update=list(si.on_update or []))
                out.append(inst)
            blk.instructions = out


def build_nc(nblk=19, repeat=1, prodw=PRODW):
    _patch_tile_drain()
    slab_rows = 4 + RPB * nblk
    npix = slab_rows * W

    nc = bass.Bass()
    x = nc.dram_tensor("x", [C, npix + XPAD], BF, kind="ExternalInput")
    labw = nc.dram_tensor("labw", [nblk, NROWS, 2 * W], BF,
                          kind="ExternalInput")
    eye = nc.dram_tensor("eye", [C, NSH, NSH], BF, kind="ExternalInput")
    eq = nc.dram_tensor("eq", [NSH, 6, NROWS], F32, kind="ExternalInput")
    out = nc.dram_tensor("out", [NROWS, nblk], F32, kind="ExternalOutput")

    nchunk = 5 * W // prodw          # chunks per wave
    cpm = prodw // W                 # rows per chunk if prodw >= W
    with TileContext(nc) as tc:
        with (tc.tile_pool(name="const", bufs=1) as cpool,
              tc.tile_pool(name="gbuf", bufs=2) as gpool,
              tc.tile_pool(name="sq", bufs=2) as sqpool,
              tc.tile_pool(name="tm", bufs=13) as tpool,
              tc.tile_pool(name="inv", bufs=2) as ipool,
              tc.tile_pool(name="pack", bufs=3) as packpool,
              tc.tile_pool(name="post", bufs=2) as postpool,
              tc.tile_pool(name="npsum", bufs=1, space="PSUM") as npsum,
              tc.tile_pool(name="dpsum", bufs=6, space="PSUM") as dpsum,
              tc.tile_pool(name="qpsum", bufs=1, space="PSUM") as qpsum):

            eye_sb = cpool.tile([C, NSH, NSH], BF)
            nc.sync.dma_start(eye_sb[:], eye[:])
            eq_sb = cpool.tile([NSH, 6, NROWS], F32)
            nc.sync.dma_start(eq_sb[:], eq[:])
            acc = cpool.tile([NROWS, nblk], F32)
            nc.vector.memset(acc[:], 0.0)

            for n in [i for _ in range(repeat) for i in range(nblk)]:
                y0 = 2 + RPB * n
                win0 = (y0 - 2) * W

                # ---- feature window: direct bf16 loads ----
                g = gpool.tile([C, GW + GPAD], BF, tag="g")
                godd = gpool.tile([C, GW + GPAD], BF, tag="godd")
                nc.sync.dma_start(g[:], x[:, win0:win0 + GW + GPAD])
                nc.sync.dma_start(godd[:], x[:, win0 + 1:win0 + GW + GPAD + 1])
                fresh0, nfr = 2 * W, 12

                # ---- norms of the 12 window rows ----
                sqt = sqpool.tile([C, 12 * W], BF, tag="sq")
                nc.scalar.square(sqt[:, 0:nfr * W],
                                 g[:, fresh0:fresh0 + nfr * W])
                n2 = npsum.tile([NSH, W], F32, tag="n2")
                for r in range(nfr):
                    nc.tensor.matmul(n2[:], eye_sb[:, r, :],
                                     sqt[:, r * W:(r + 1) * W],
                                     start=(r == 0), stop=(r == nfr - 1))
                lnt = ipool.tile([NSH, W + 4], F32, tag="lnt")
                nc.scalar.activation(lnt[:, 2:W + 2], n2[:],
                                     mybir.ActivationFunctionType.Ln)
                if n <= 1:
                    nc.vector.memset(lnt[:, 0:2], 1e30)
                    nc.vector.memset(lnt[:, W + 2:W + 4], 1e30)

                # ---- dots ----
                pk = postpool.tile([NROWS, W], BF, tag="pk")
                s_all = packpool.tile([NSH, RPB * W], BF, tag="sall")
                base0 = 2 * W
                ts = []
                for m in range(NSH):
                    off = OFFS[m]
                    t = tpool.tile([C, 10 * W], BF, tag="t")
                    in0 = g[:, base0:base0 + 10 * W]
                    if off % 2 == 0:
                        in1 = g[:, base0 + off:base0 + off + 10 * W]
                    else:
                        in1 = godd[:, base0 + off - 1:
                                   base0 + off - 1 + 10 * W]
                    nc.vector.tensor_mul(t[:], in0, in1)
                    ts.append(t)
                for w in range(2):
                    pd = [dpsum.tile([NSH, W], F32, tag="pd", name=f"pd{w}_{i}")
                          for i in range(5)]
                    for m in range(NSH):
                        t = ts[m]
                        for r5 in range(5):
                            nc.tensor.matmul(
                                pd[r5][:], eye_sb[:, m, :],
                                t[:, (w * 5 + r5) * W:(w * 5 + r5 + 1) * W],
                                start=(m == 0), stop=(m == NSH - 1))
                    for r5 in range(5):
                        nc.scalar.copy(s_all[:, (w * 5 + r5) * W:
                                             (w * 5 + r5 + 1) * W],
                                       pd[r5][:])

                # one DMA: pk row p' = 10*m + r  <-  s_all[m, r*W + x]
                nc.sync.dma_start(
                    pk[:], s_all[:].rearrange("m (r x) -> m r x", r=RPB))

                # ---- fused inv-product: ln n2_p + ln n2_q -> exp(-0.5 .) ----
                iqp = qpsum.tile([NROWS, W], F32, tag="iqp")
                nc.tensor.matmul(iqp[:], eq_sb[:, 0, :], lnt[:, 2:W + 2],
                                 start=True, stop=False)
                for di in range(5):
                    dx = di - 2
                    nc.tensor.matmul(iqp[:], eq_sb[:, 1 + di, :],
                                     lnt[:, 2 + dx:2 + dx + W],
                                     start=False, stop=(di == 4))
                ipq = postpool.tile([NROWS, W], BF, tag="ipq")
                nc.scalar.activation(ipq[:], iqp[:],
                                     mybir.ActivationFunctionType.Exp,
                                     scale=-0.5)

                lw = postpool.tile([NROWS, 2 * W], BF, tag="lw")
                nc.sync.dma_start(lw[:], labw[n])
                lab = lw[:, 0:W]
                ww = lw[:, W:2 * W]

                # ---- post ----
                u2 = postpool.tile([NROWS, W], BF, tag="u2")
                u3 = postpool.tile([NROWS, W], BF, tag="u3")
                u4 = postpool.tile([NROWS, W], BF, tag="u4")
                u5 = postpool.tile([NROWS, W], BF, tag="u5")
                nc.vector.tensor_mul(u2[:], pk[:], ipq[:])
                nc.vector.tensor_sub(u3[:], u2[:], lab)
                nc.vector.tensor_mul(u4[:], u3[:], ww)
                nc.vector.scalar_tensor_tensor(
                    u5[:], u4[:], 1.0, u3[:],
                    mybir.AluOpType.mult, mybir.AluOpType.mult,
                    accum_out=acc[:, n:n + 1])

            nc.sync.dma_start(out[:], acc[:])
    _split_multi_waits(nc)
    return nc


def make_consts():
    eye = np.broadcast_to(np.eye(NSH, dtype=BF16), (C, NSH, NSH)).copy()
    eq = np.zeros((6, NSH, NROWS), np.float32)
    for r in range(RPB):
        for m in range(NSH):
            eq[0, r, RPB * m + r] = 1
    for m, (dy, dx) in enumerate(SHIFTS):
        for r in range(RPB):
            eq[1 + (dx + 2), r + dy, RPB * m + r] = 1
    return eye, np.ascontiguousarray(eq.transpose(1, 0, 2))


def host_prep(er_input, seg_label, gt_boundary_seg, nblk=19):
    B, _, H, Wd_ = er_input.shape
    f32 = np.float32
    gb = np.where(gt_boundary_seg == 255, 0, gt_boundary_seg)
    slc = np.where(seg_label == 255, 0, seg_label)
    gt_b1 = gb * slc[:, 1]
    boundary = gt_b1 > 0
    iy = np.arange(H)
    ix = np.arange(Wd_)
    interior = (((iy >= 2) & (iy <= H - 3))[:, None]
                & ((ix >= 2) & (ix <= Wd_ - 3))[None, :])
    sel = boundary & interior
    cnt = sel.sum(axis=(1, 2)).astype(f32)
    valid = boundary.sum(axis=(1, 2)) >= 1
    n_valid = valid.astype(f32).sum()

    seg_f = seg_label.astype(f32)
    lab_stack = np.empty((NSH, B, H, Wd_), f32)
    w_stack = np.empty((NSH, B, H, Wd_), f32)
    sel_f = sel.astype(f32)
    for m, (dy, dx) in enumerate(SHIFTS):
        rolled = np.roll(seg_f, (-dy, -dx), axis=(2, 3))
        lab_stack[m] = (seg_f * rolled).sum(axis=1)
        sh = np.zeros_like(sel_f)
        ys0, ys1 = max(0, -dy), min(H, H - dy)
        xs0, xs1 = max(0, -dx), min(Wd_, Wd_ - dx)
        sh[:, ys0:ys1, xs0:xs1] = sel_f[:, ys0 + dy:ys1 + dy,
                                        xs0 + dx:xs1 + dx]
        w_stack[m] = sel_f + sh

    eye, eq = make_consts()
    slab_rows = 4 + RPB * nblk
    npix = slab_rows * Wd_
    per_core = []
    for k in range(8):
        b, h = k // 2, k % 2
        r0 = 0 if h == 0 else 190
        xs = np.zeros((C, npix + XPAD), BF16)
        xs[:, 0:npix] = (er_input[b, :, r0:r0 + slab_rows, :]
                         .reshape(C, -1).astype(BF16))
        rows = r0 + 2 + np.arange(RPB * nblk)
        labc = lab_stack[:, b, rows, :].reshape(NSH, nblk, RPB, Wd_)
        wc = w_stack[:, b, rows, :].reshape(NSH, nblk, RPB, Wd_)
        labc = labc.transpose(1, 0, 2, 3).reshape(nblk, NROWS, Wd_)
        wc = wc.transpose(1, 0, 2, 3).reshape(nblk, NROWS, Wd_)
        lw = np.stack([labc, wc], axis=2).astype(BF16).reshape(
            nblk, NROWS, 2 * Wd_)
        per_core.append({"x": xs, "labw": lw, "eye": eye, "eq": eq})
    return per_core, dict(cnt=cnt, valid=valid, n_valid=n_valid)


def finish(core_sums, meta):
    f32 = np.float32
    cnt, valid, n_valid = meta["cnt"], meta["valid"], meta["n_valid"]
    total = f32(0.0)
    for b in range(4):
        sb = f32(core_sums[2 * b] + core_sums[2 * b + 1])
        loss_b = sb / max(cnt[b], f32(1.0)) / f32(24.0)
        if valid[b]:
            total = total + loss_b
    total = total / max(n_valid, f32(1.0))
    if np.isnan(total):
        total = f32(0.0)
    return np.float32(total)


_NC_CACHE = {}


def kernel(er_input, seg_label, gt_boundary_seg):
    er_input = np.asarray(er_input)
    seg_label = np.asarray(seg_label)
    gt_boundary_seg = np.asarray(gt_boundary_seg)
    per_core, meta = host_prep(er_input, seg_label, gt_boundary_seg)
    if "nc" not in _NC_CACHE:
        _NC_CACHE["nc"] = build_nc()
    nc = _NC_CACHE["nc"]
    res = bass_utils.run_bass_kernel_spmd(nc, per_core,
                                          core_ids=list(range(8)))
    sums = [r["out"].astype(np.float64).sum() for r in res.results]
    return finish(sums, meta)


# revision 18
# speedup vs baseline: 1.1688x; 1.1688x over previous
"""Bass TRN2 kernel for the boundary cosine-similarity context loss.

Per core (8 cores): batch b = k//2, row-half h = k%2; slab = 194 rows
(h==0: global rows 0..193, h==1: 190..383); produced = slab rows 2..191.
19 macroblocks x 10 produced rows.

v2 layout: host supplies x as bf16 (padded); per block the 14-row g
window and its +1-shifted twin godd are DMA'd straight from HBM (no
gpsimd conversion, no inter-block copy chain). Dot products are chunked
[C, PRODW] on DVE feeding one-hot reduction matmuls in PSUM; norms are
ACT square + eye matmuls -> ln -> fused inv-product psum (identity +
5 dx-shift selection matmuls accumulate ln n2_p + ln n2_q) -> exp ->
ipq. Post: cos = pk*ipq; TTR-fused weighted MSE accumulation.
Host folds the per-batch 1/cnt, 1/24, valid, 1/n_valid scaling.
"""

import numpy as np
import ml_dtypes

import concourse.bass as bass
import concourse.mybir as mybir
from concourse.tile import TileContext
from concourse.vector_clock import ScopedClock
from concourse import bass_utils

BF16 = ml_dtypes.bfloat16
F32 = mybir.dt.float32
BF = mybir.dt.bfloat16

W = 384
C = 128
SHIFTS = [(0, 1), (0, 2),
          (1, -2), (1, -1), (1, 0), (1, 1), (1, 2),
          (2, -2), (2, -1), (2, 0), (2, 1), (2, 2)]
OFFS = [dy * W + dx for dy, dx in SHIFTS]
NSH = 12
RPB = 10
NROWS = NSH * RPB      # 120 packed rows
GW = 14 * W
GPAD = 8
XPAD = 16
PRODW = 384            # product chunk width (per DVE op)


def _patch_tile_drain():
    if getattr(TileContext, "_drain_patched", False):
        return

    def _drain_and_barrier(self, tick_clock, wait_clock):
        drain_inst = self.nc.sync.drain()
        wait_clock.add_sem_waits(
            drain_inst.ins, ScopedClock({None: tick_clock.global_clock}))
        si = drain_inst.ins.sync_info
        if si is not None and si.on_wait and len(si.on_wait) > 1:
            waits = list(si.on_wait)
            drain_inst.ins.sync_info = mybir.SyncInfo(
                on_wait=[waits[-1]], on_update=list(si.on_update or []))
            for w in waits[:-1]:
                nop = self.nc.sync.nop(nofuse=True)
                nop.ins.sync_info = mybir.SyncInfo(on_wait=[w], on_update=[])
        self.nc.all_engine_barrier()
        popped = self.nc._tile_sem_poison_stack.pop()
        assert popped is self._sem_poison
        self.nc.clear_and_free_semaphores(list(self.sems.allocated().values()))
        self.nc.all_engine_barrier()

    TileContext._drain_and_barrier = _drain_and_barrier
    TileContext._drain_patched = True


_WSPLIT_N = [0]


def _split_multi_waits(nc, max_waits=1):
    """This container's walrus rejects instructions with more than one sync
    wait; hoist excess waits onto same-engine NOPs inserted just before."""
    for fn in nc.m.functions:
        for blk in fn.blocks:
            insts = blk.instructions
            out = []
            for inst in insts:
                si = inst.sync_info
                if si is not None and si.on_wait and len(si.on_wait) > max_waits:
                    waits = list(si.on_wait)
                    keep = waits[-max_waits:]
                    for w in waits[:-max_waits]:
                        _WSPLIT_N[0] += 1
                        nop = mybir.InstNoOp(
                            name=f"wsplit_{_WSPLIT_N[0]}", ins=[], outs=[])
                        nop.engine = inst.engine
                        nop.sync_info = mybir.SyncInfo(on_wait=[w],
                                                       on_update=[])
                        out.append(nop)
                    inst.sync_info = mybir.SyncInfo(
                        on_wait=keep, on_update=list(si.on_update or []))
                out.append(inst)
            blk.instructions = out


def build_nc(nblk=19, repeat=1, prodw=PRODW):
    _patch_tile_drain()
    slab_rows = 4 + RPB * nblk
    npix = slab_rows * W

    nc = bass.Bass()
    x = nc.dram_tensor("x", [C, npix + XPAD], BF, kind="ExternalInput")
    labw = nc.dram_tensor("labw", [nblk, NROWS, 2 * W], BF,
                          kind="ExternalInput")
    eye = nc.dram_tensor("eye", [C, NSH, NSH], BF, kind="ExternalInput")
    eq = nc.dram_tensor("eq", [NSH, 6, NROWS], F32, kind="ExternalInput")
    out = nc.dram_tensor("out", [NROWS, nblk], F32, kind="ExternalOutput")

    nchunk = 5 * W // prodw          # chunks per wave
    cpm = prodw // W                 # rows per chunk if prodw >= W
    with TileContext(nc) as tc:
        with (tc.tile_pool(name="const", bufs=1) as cpool,
              tc.tile_pool(name="gbuf", bufs=2) as gpool,
              tc.tile_pool(name="sq", bufs=2) as sqpool,
              tc.tile_pool(name="tm", bufs=13) as tpool,
              tc.tile_pool(name="inv", bufs=2) as ipool,
              tc.tile_pool(name="pack", bufs=3) as packpool,
              tc.tile_pool(name="post", bufs=2) as postpool,
              tc.tile_pool(name="npsum", bufs=2, space="PSUM") as npsum,
              tc.tile_pool(name="dpsum", bufs=5, space="PSUM") as dpsum,
              tc.tile_pool(name="qpsum", bufs=1, space="PSUM") as qpsum):

            eye_sb = cpool.tile([C, NSH, NSH], BF)
            nc.sync.dma_start(eye_sb[:], eye[:])
            eq_sb = cpool.tile([NSH, 6, NROWS], F32)
            nc.sync.dma_start(eq_sb[:], eq[:])
            acc = cpool.tile([NROWS, nblk], F32)
            nc.vector.memset(acc[:], 0.0)

            for n in [i for _ in range(repeat) for i in range(nblk)]:
                y0 = 2 + RPB * n
                win0 = (y0 - 2) * W

                # ---- feature window: direct bf16 loads ----
                g = gpool.tile([C, GW + GPAD], BF, tag="g")
                godd = gpool.tile([C, GW + GPAD], BF, tag="godd")
                nc.sync.dma_start(g[:], x[:, win0:win0 + GW + GPAD])
                nc.sync.dma_start(godd[:], x[:, win0 + 1:win0 + GW + GPAD + 1])
                fresh0, nfr = 2 * W, 12

                # ---- norms of the 12 window rows ----
                sqt = sqpool.tile([C, 12 * W], BF, tag="sq")
                nc.scalar.square(sqt[:, 0:nfr * W],
                                 g[:, fresh0:fresh0 + nfr * W])
                n2 = npsum.tile([NSH, W], F32, tag="n2")
                for r in range(nfr):
                    nc.tensor.matmul(n2[:], eye_sb[:, r, :],
                                     sqt[:, r * W:(r + 1) * W],
                                     start=(r == 0), stop=(r == nfr - 1))
                lnt = ipool.tile([NSH, W + 4], F32, tag="lnt")
                nc.scalar.activation(lnt[:, 2:W + 2], n2[:],
                                     mybir.ActivationFunctionType.Ln)
                if n <= 1:
                    nc.vector.memset(lnt[:, 0:2], 1e30)
                    nc.vector.memset(lnt[:, W + 2:W + 4], 1e30)

                # ---- dots ----
                pk = postpool.tile([NROWS, W], BF, tag="pk")
                s_all = packpool.tile([NSH, RPB * W], BF, tag="sall")
                base0 = 2 * W
                ts = []
                for m in range(NSH):
                    off = OFFS[m]
                    t = tpool.tile([C, 10 * W], BF, tag="t")
                    in0 = g[:, base0:base0 + 10 * W]
                    if off % 2 == 0:
                        in1 = g[:, base0 + off:base0 + off + 10 * W]
                    else:
                        in1 = godd[:, base0 + off - 1:
                                   base0 + off - 1 + 10 * W]
                    nc.vector.tensor_mul(t[:], in0, in1)
                    ts.append(t)
                for w in range(2):
                    pd = [dpsum.tile([NSH, W], F32, tag="pd", name=f"pd{w}_{i}")
                          for i in range(5)]
                    for m in range(NSH):
                        t = ts[m]
                        for r5 in range(5):
                            nc.tensor.matmul(
                                pd[r5][:], eye_sb[:, m, :],
                                t[:, (w * 5 + r5) * W:(w * 5 + r5 + 1) * W],
                                start=(m == 0), stop=(m == NSH - 1))
                    for r5 in range(5):
                        nc.scalar.copy(s_all[:, (w * 5 + r5) * W:
                                             (w * 5 + r5 + 1) * W],
                                       pd[r5][:])

                # one DMA: pk row p' = 10*m + r  <-  s_all[m, r*W + x]
                nc.sync.dma_start(
                    pk[:], s_all[:].rearrange("m (r x) -> m r x", r=RPB))

                # ---- fused inv-product: ln n2_p + ln n2_q -> exp(-0.5 .) ----
                iqp = qpsum.tile([NROWS, W], F32, tag="iqp")
                nc.tensor.matmul(iqp[:], eq_sb[:, 0, :], lnt[:, 2:W + 2],
                                 start=True, stop=False)
                for di in range(5):
                    dx = di - 2
                    nc.tensor.matmul(iqp[:], eq_sb[:, 1 + di, :],
                                     lnt[:, 2 + dx:2 + dx + W],
                                     start=False, stop=(di == 4))
                ipq = postpool.tile([NROWS, W], BF, tag="ipq")
                nc.scalar.activation(ipq[:], iqp[:],
                                     mybir.ActivationFunctionType.Exp,
                                     scale=-0.5)

                lw = postpool.tile([NROWS, 2 * W], BF, tag="lw")
                nc.sync.dma_start(lw[:], labw[n])
                lab = lw[:, 0:W]
                ww = lw[:, W:2 * W]

                # ---- post ----
                u2 = postpool.tile([NROWS, W], BF, tag="u2")
                u3 = postpool.tile([NROWS, W], BF, tag="u3")
                u4 = postpool.tile([NROWS, W], BF, tag="u4")
                u5 = postpool.tile([NROWS, W], BF, tag="u5")
                nc.vector.tensor_mul(u2[:], pk[:], ipq[:])
                nc.vector.tensor_sub(u3[:], u2[:], lab)
                nc.vector.tensor_mul(u4[:], u3[:], ww)
                nc.vector.scalar_tensor_tensor(
                    u5[:], u4[:], 1.0, u3[:],
                    mybir.AluOpType.mult, mybir.AluOpType.mult,
                    accum_out=acc[:, n:n + 1])

            nc.sync.dma_start(out[:], acc[:])
    _split_multi_waits(nc)
    return nc


def make_consts():
    eye = np.broadcast_to(np.eye(NSH, dtype=BF16), (C, NSH, NSH)).copy()
    eq = np.zeros((6, NSH, NROWS), np.float32)
    for r in range(RPB):
        for m in range(NSH):
            eq[0, r, RPB * m + r] = 1
    for m, (dy, dx) in enumerate(SHIFTS):
        for r in range(RPB):
            eq[1 + (dx + 2), r + dy, RPB * m + r] = 1
    return eye, np.ascontiguousarray(eq.transpose(1, 0, 2))


def host_prep(er_input, seg_label, gt_boundary_seg, nblk=19):
    B, _, H, Wd_ = er_input.shape
    f32 = np.float32
    gb = np.where(gt_boundary_seg == 255, 0, gt_boundary_seg)
    slc = np.where(seg_label == 255, 0, seg_label)
    gt_b1 = gb * slc[:, 1]
    boundary = gt_b1 > 0
    iy = np.arange(H)
    ix = np.arange(Wd_)
    interior = (((iy >= 2) & (iy <= H - 3))[:, None]
                & ((ix >= 2) & (ix <= Wd_ - 3))[None, :])
    sel = boundary & interior
    cnt = sel.sum(axis=(1, 2)).astype(f32)
    valid = boundary.sum(axis=(1, 2)) >= 1
    n_valid = valid.astype(f32).sum()

    seg_f = seg_label.astype(f32)
    lab_stack = np.empty((NSH, B, H, Wd_), f32)
    w_stack = np.empty((NSH, B, H, Wd_), f32)
    sel_f = sel.astype(f32)
    for m, (dy, dx) in enumerate(SHIFTS):
        rolled = np.roll(seg_f, (-dy, -dx), axis=(2, 3))
        lab_stack[m] = (seg_f * rolled).sum(axis=1)
        sh = np.zeros_like(sel_f)
        ys0, ys1 = max(0, -dy), min(H, H - dy)
        xs0, xs1 = max(0, -dx), min(Wd_, Wd_ - dx)
        sh[:, ys0:ys1, xs0:xs1] = sel_f[:, ys0 + dy:ys1 + dy,
                                        xs0 + dx:xs1 + dx]
        w_stack[m] = sel_f + sh

    eye, eq = make_consts()
    slab_rows = 4 + RPB * nblk
    npix = slab_rows * Wd_
    per_core = []
    for k in range(8):
        b, h = k // 2, k % 2
        r0 = 0 if h == 0 else 190
        xs = np.zeros((C, npix + XPAD), BF16)
        xs[:, 0:npix] = (er_input[b, :, r0:r0 + slab_rows, :]
                         .reshape(C, -1).astype(BF16))
        rows = r0 + 2 + np.arange(RPB * nblk)
        labc = lab_stack[:, b, rows, :].reshape(NSH, nblk, RPB, Wd_)
        wc = w_stack[:, b, rows, :].reshape(NSH, nblk, RPB, Wd_)
        labc = labc.transpose(1, 0, 2, 3).reshape(nblk, NROWS, Wd_)
        wc = wc.transpose(1, 0, 2, 3).reshape(nblk, NROWS, Wd_)
        lw = np.stack([labc, wc], axis=2).astype(BF16).reshape(
            nblk, NROWS, 2 * Wd_)
        per_core.append({"x": xs, "labw": lw, "eye": eye, "eq": eq})
    return per_core, dict(cnt=cnt, valid=valid, n_valid=n_valid)


def finish(core_sums, meta):
    f32 = np.float32
    cnt, valid, n_valid = meta["cnt"], meta["valid"], meta["n_valid"]
    total = f32(0.0)
    for b in range(4):
        sb = f32(core_sums[2 * b] + core_sums[2 * b + 1])
        loss_b = sb / max(cnt[b], f32(1.0)) / f32(24.0)
        if valid[b]:
            total = total + loss_b
    total = total / max(n_valid, f32(1.0))
    if np.isnan(total):
        total = f32(0.0)
    return np.float32(total)


_NC_CACHE = {}


def kernel(er_input, seg_label, gt_boundary_seg):
    er_input = np.asarray(er_input)
    seg_label = np.asarray(seg_label)
    gt_boundary_seg = np.asarray(gt_boundary_seg)
    per_core, meta = host_prep(er_input, seg_label, gt_boundary_seg)
    if "nc" not in _NC_CACHE:
        _NC_CACHE["nc"] = build_nc()
    nc = _NC_CACHE["nc"]
    res = bass_utils.run_bass_kernel_spmd(nc, per_core,
                                          core_ids=list(range(8)))
    sums = [r["out"].astype(np.float64).sum() for r in res.results]
    return finish(sums, meta)
